# revision 6
# baseline (speedup 1.0000x reference)
"""Trainium2 Bass kernel for nn_Decoder (LSTM-cell decoder + dot-product
attention + tied-embedding projection), data-parallel over batch on 8 cores.

Contract: kernel(**inputs) takes the FULL unsharded numpy inputs (as produced
by the reference setup_inputs) and returns the FULL (B, VOCAB, T) fp32 output.

Strategy per core (4 batches each):
  - host: shift y, gather embeddings, transpose to (EMBED, ntok) layout
  - device: LSTM1/LSTM2 (zero-state cells; f-gate skipped), dot-product
    attention with length masking (mask added via K=1 ones-matmul into the
    PSUM accumulation group), softmax on free dim, PE-transpose of attn,
    context matmul, then the dominant (EMBED x VOCAB) tied projection with
    E^T streamed from HBM; all matmuls run as float32r (full-rate fp32).
"""
import sys, os

for _p in ("/opt/trn_rl_repo",):
    if _p not in sys.path and os.path.isdir(_p):
        sys.path.insert(0, _p)

import numpy as np

VOCAB = 32000
EMBED = 256
H1 = 512
KV = 128
B = 32
S = 512
T = 256
N_CORES = 8
BPC = B // N_CORES          # 4 batches per core
NTOK = BPC * T              # 1024 tokens per core
NVT = VOCAB // 128          # 250 vocab tiles
VCHUNK = 8                  # vocab tiles per E-stream chunk
NCHUNK = (NVT + VCHUNK - 1) // VCHUNK  # 32 (last chunk has 2 tiles)

# Logits-matmul dtype for the tied-projection phase. "f16" streams E^T as
# fp16 (h2/ctx cast to fp16): halves the dominant E-read DMA at full PE
# rate, and fp16's 11-bit mantissa keeps the end-to-end error ~3e-4
# (vs 2.2e-4 all-fp32r, 2.4e-3 bf16). "f32r" is the full-precision path.
LOGITS_DT = os.environ.get("KERNEL_LOGITS_DT", "f16")
LOGITS_BF16 = LOGITS_DT in ("bf16", "f16")
# Output store dtype. "f16" halves the dominant 131MB/core logits store
# (quantization adds ~1e-4 rel err, well under the 2e-2 gate); host widens
# back to fp32. Device layout is (VOCAB, BPC*T) so stores are contiguous;
# host transposes to (BPC, VOCAB, T).
OUT_DT = os.environ.get("KERNEL_OUT_DT", "f16")

_COMPILED = None


def _build_module():
    import concourse.tile as tile
    import concourse.mybir as mybir
    from concourse import bacc, masks

    F32 = mybir.dt.float32
    F32R = mybir.dt.float32r
    BF16 = (mybir.dt.float16 if LOGITS_DT == "f16" else mybir.dt.bfloat16)
    EDT = BF16 if LOGITS_BF16 else F32R
    ODT = mybir.dt.float16 if OUT_DT == "f16" else F32
    AF = mybir.ActivationFunctionType
    AX = mybir.AxisListType
    ALU = mybir.AluOpType

    nc = bacc.Bacc("TRN2", target_bir_lowering=False, debug=False,
                   num_devices=N_CORES)

    def din(name, shape, dt=F32):
        return nc.dram_tensor(name, list(shape), dt, kind="ExternalInput").ap()

    xT_d = din("xT", (EMBED, NTOK), F32R)      # embedded tokens, transposed
    keyT_d = din("keyT", (BPC, KV, S), F32R)   # key[b].T
    val_d = din("val", (BPC, S, KV), F32R)     # value[b]
    mask_d = din("maskb", (BPC, S), F32R)      # 0 / -1e9 additive mask
    w1T_d = din("w1T", (EMBED, 3 * H1), F32R)  # W_ih1[i,g,o].T
    b1_d = din("b1", (128, 12))                # (b_ih1+b_hh1)[i,g,o] tiled
    w2T_d = din("w2T", (H1, 3 * KV), F32R)     # W_ih2[i,g,o].T
    b2_d = din("b2", (128, 3))
    ET_d = din("ET", (EMBED, VOCAB), EDT)      # E.T
    bout_d = din("bout", (128, NVT))           # b_out tiled
    ones_d = din("ones1", (1, 128), F32R)
    ident_d = din("ident", (128, 128), F32R)
    out_d = nc.dram_tensor("out", [VOCAB, NTOK], ODT,
                           kind="ExternalOutput").ap()

    with tile.TileContext(nc) as tc:
        import contextlib
        ctx = contextlib.ExitStack()
        with ctx:
            sb = ctx.enter_context(tc.tile_pool(name="sb", bufs=1))
            work = ctx.enter_context(tc.tile_pool(name="work", bufs=3))
            epool = ctx.enter_context(tc.tile_pool(name="epool", bufs=6))
            opool = ctx.enter_context(tc.tile_pool(name="opool", bufs=6))
            ps512 = ctx.enter_context(
                tc.tile_pool(name="ps512", bufs=4, space="PSUM"))
            pstr = ctx.enter_context(
                tc.tile_pool(name="pstr", bufs=2, space="PSUM"))
            psctx = ctx.enter_context(
                tc.tile_pool(name="psctx", bufs=2, space="PSUM"))

            # ---------------- resident inputs ----------------
            xT = [sb.tile([128, NTOK], F32R, name=f"xT{k}") for k in range(2)]
            for k in range(2):
                nc.sync.dma_start(xT[k][:], xT_d[k * 128:(k + 1) * 128, :])
            w1T = [sb.tile([128, 3 * H1], F32R, name=f"w1T{k}") for k in range(2)]
            for k in range(2):
                nc.sync.dma_start(w1T[k][:], w1T_d[k * 128:(k + 1) * 128, :])
            w2T = [sb.tile([128, 3 * KV], F32R, name=f"w2T{k}") for k in range(4)]
            for k in range(4):
                nc.sync.dma_start(w2T[k][:], w2T_d[k * 128:(k + 1) * 128, :])
            b1 = sb.tile([128, 12], F32)
            nc.sync.dma_start(b1[:], b1_d[:])
            b2 = sb.tile([128, 3], F32)
            nc.sync.dma_start(b2[:], b2_d[:])
            keyT = sb.tile([128, BPC * S], F32R)
            for b in range(BPC):
                nc.sync.dma_start(keyT[:, b * S:(b + 1) * S], keyT_d[b])
            valt = sb.tile([128, BPC * S], F32R)  # value k-tiles side by side
            for b in range(BPC):
                nc.sync.dma_start(
                    valt[:, b * S:(b + 1) * S].rearrange(
                        "p (st v) -> p st v", st=4),
                    val_d[b].rearrange("(st p) v -> p st v", p=128))
            maskb = sb.tile([1, BPC * S], F32R)
            nc.sync.dma_start(
                maskb[0:1, :].rearrange("p (b s) -> p b s", b=BPC),
                mask_d[None])
            bout = sb.tile([128, NVT], F32)
            nc.sync.dma_start(bout[:], bout_d[:])
            ones1 = sb.tile([1, 128], F32R)
            nc.sync.dma_start(ones1[:], ones_d[:])
            ident = sb.tile([128, 128], F32R)
            nc.sync.dma_start(ident[:], ident_d[:])

            h1T = [sb.tile([128, NTOK], F32R, name=f"h1T{k}") for k in range(4)]
            h2T = sb.tile([128, NTOK], F32R)
            ctxT = sb.tile([128, NTOK], EDT)
            h2L = (sb.tile([128, NTOK], BF16, name="h2L")
                   if LOGITS_BF16 else h2T)

            def r(ap):
                return ap if ap.dtype == F32R else ap.bitcast(F32R)

            # ---------------- LSTM 1 ----------------
            # gates^T tiles: m = 0..3 -> i, 4..7 -> g, 8..11 -> o
            for n in range(2):
                tok = slice(n * 512, (n + 1) * 512)
                for msub in range(4):
                    pg = {}
                    for gi, gname in enumerate(("i", "g", "o")):
                        m = gi * 4 + msub
                        ps = ps512.tile([128, 512], F32, name="ps_mm",
                                        tag="ps_mm")
                        for k in range(2):
                            nc.tensor.matmul(
                                ps[:],
                                r(w1T[k][:, m * 128:(m + 1) * 128]),
                                r(xT[k][:, tok]),
                                start=(k == 0), stop=(k == 1))
                        pg[gname] = (ps, m)
                    sig_i = work.tile([128, 512], F32, tag="lstm_act")
                    nc.scalar.activation(sig_i[:], pg["i"][0][:], AF.Sigmoid,
                                         bias=b1[:, pg["i"][1]:pg["i"][1] + 1])
                    tanh_g = work.tile([128, 512], F32, tag="lstm_act")
                    nc.scalar.activation(tanh_g[:], pg["g"][0][:], AF.Tanh,
                                         bias=b1[:, pg["g"][1]:pg["g"][1] + 1])
                    cst = work.tile([128, 512], F32, tag="lstm_act")
                    nc.vector.tensor_mul(cst[:], sig_i[:], tanh_g[:])
                    tanh_c = work.tile([128, 512], F32, tag="lstm_act")
                    nc.scalar.activation(tanh_c[:], cst[:], AF.Tanh)
                    sig_o = work.tile([128, 512], F32, tag="lstm_act")
                    nc.scalar.activation(sig_o[:], pg["o"][0][:], AF.Sigmoid,
                                         bias=b1[:, pg["o"][1]:pg["o"][1] + 1])
                    nc.vector.tensor_mul(h1T[msub][:, tok], sig_o[:], tanh_c[:])

            # ---------------- LSTM 2 ----------------
            for n in range(2):
                tok = slice(n * 512, (n + 1) * 512)
                pg = {}
                for gi, gname in enumerate(("i", "g", "o")):
                    ps = ps512.tile([128, 512], F32, name="ps_mm", tag="ps_mm")
                    for k in range(4):
                        nc.tensor.matmul(
                            ps[:],
                            r(w2T[k][:, gi * 128:(gi + 1) * 128]),
                            r(h1T[k][:, tok]),
                            start=(k == 0), stop=(k == 3))
                    pg[gname] = ps
                sig_i = work.tile([128, 512], F32, tag="lstm_act")
                nc.scalar.activation(sig_i[:], pg["i"][:], AF.Sigmoid,
                                     bias=b2[:, 0:1])
                tanh_g = work.tile([128, 512], F32, tag="lstm_act")
                nc.scalar.activation(tanh_g[:], pg["g"][:], AF.Tanh,
                                     bias=b2[:, 1:2])
                cst = work.tile([128, 512], F32, tag="lstm_act")
                nc.vector.tensor_mul(cst[:], sig_i[:], tanh_g[:])
                tanh_c = work.tile([128, 512], F32, tag="lstm_act")
                nc.scalar.activation(tanh_c[:], cst[:], AF.Tanh)
                sig_o = work.tile([128, 512], F32, tag="lstm_act")
                nc.scalar.activation(sig_o[:], pg["o"][:], AF.Sigmoid,
                                     bias=b2[:, 2:3])
                nc.vector.tensor_mul(h2T[:, tok], sig_o[:], tanh_c[:])
                if LOGITS_BF16:
                    nc.vector.tensor_copy(h2L[:, tok], h2T[:, tok])

            # ---------------- attention ----------------
            for b in range(BPC):
                attnT = [work.tile([128, T], F32R, tag=f"attnT{st}",
                                   name=f"attnT{st}")
                         for st in range(4)]
                for tt in range(2):
                    tcol = b * T + tt * 128
                    ps_e = ps512.tile([128, 512], F32, name="ps_mm",
                                      tag="ps_mm")
                    nc.tensor.matmul(ps_e[:],
                                     r(h2T[:, tcol:tcol + 128]),
                                     r(keyT[:, b * S:(b + 1) * S]),
                                     start=True, stop=False)
                    nc.tensor.matmul(ps_e[:], r(ones1[:]),
                                     r(maskb[0:1, b * S:(b + 1) * S]),
                                     start=False, stop=True)
                    negmax = work.tile([128, 1], F32, tag="stat")
                    nc.vector.tensor_reduce(negmax[:], ps_e[:], axis=AX.X,
                                            op=ALU.max, negate=True)
                    attn = work.tile([128, 512], F32, tag="attn")
                    rowsum = work.tile([128, 1], F32, tag="stat")
                    nc.scalar.activation(attn[:], ps_e[:], AF.Exp,
                                         bias=negmax[:, 0:1],
                                         accum_out=rowsum[:, 0:1])
                    recip = work.tile([128, 1], F32, tag="stat")
                    nc.vector.reciprocal(recip[:], rowsum[:])
                    attn_n = work.tile([128, 512], F32R, tag="attn_n")
                    nc.vector.tensor_scalar_mul(attn_n[:], attn[:],
                                                recip[:, 0:1])
                    for st in range(4):
                        ps_t = pstr.tile([128, 128], F32R, name="ps_tr",
                                         tag="ps_tr")
                        nc.tensor.transpose(ps_t[:],
                                            r(attn_n[:, st * 128:(st + 1) * 128]),
                                            r(ident[:]))
                        dst = attnT[st][:, tt * 128:(tt + 1) * 128]
                        if st % 2 == 0:
                            nc.scalar.copy(dst, ps_t[:])
                        else:
                            nc.vector.tensor_copy(dst, ps_t[:])
                ps_c = psctx.tile([128, T], F32, name="ps_ctx", tag="ps_ctx")
                for st in range(4):
                    nc.tensor.matmul(
                        ps_c[:],
                        r(valt[:, (b * 4 + st) * 128:(b * 4 + st + 1) * 128]),
                        r(attnT[st][:]),
                        start=(st == 0), stop=(st == 3))
                if b % 2 == 0:
                    nc.scalar.copy(ctxT[:, b * T:(b + 1) * T], ps_c[:])
                else:
                    nc.vector.tensor_copy(ctxT[:, b * T:(b + 1) * T], ps_c[:])

            # ---------------- logits: out = [h2; ctx]^T . [E_lo; E_hi] ----
            for ci in range(NCHUNK):
                nv = min(VCHUNK, NVT - ci * VCHUNK)
                cols = nv * 128
                base = ci * VCHUNK * 128
                ea = epool.tile([128, VCHUNK * 128], EDT, tag="ea")
                eb = epool.tile([128, VCHUNK * 128], EDT, tag="eb")
                nc.scalar.dma_start(ea[:, :cols],
                                    ET_d[0:128, base:base + cols])
                nc.scalar.dma_start(eb[:, :cols],
                                    ET_d[128:256, base:base + cols])
                for j in range(nv):
                    v = ci * VCHUNK + j
                    osb = opool.tile([128, NTOK], ODT, tag="osb")
                    for half in range(2):
                        tok = slice(half * 512, (half + 1) * 512)
                        ps_l = ps512.tile([128, 512], F32, name="ps_mm",
                                          tag="ps_mm")
                        nc.tensor.matmul(ps_l[:],
                                         ea[:, j * 128:(j + 1) * 128],
                                         h2L[:, tok],
                                         start=True, stop=False)
                        nc.tensor.matmul(ps_l[:],
                                         eb[:, j * 128:(j + 1) * 128],
                                         ctxT[:, tok],
                                         start=False, stop=True)
                        if half == 0:
                            nc.scalar.activation(osb[:, tok], ps_l[:],
                                                 AF.Identity,
                                                 bias=bout[:, v:v + 1])
                        else:
                            nc.vector.tensor_scalar_add(osb[:, tok], ps_l[:],
                                                        bout[:, v:v + 1])
                    nc.sync.dma_start(out_d[v * 128:(v + 1) * 128, :],
                                      osb[:])

    nc.compile()
    return nc


def _prep_inputs(key, value, encoder_len, y, E, W_ih1, b_ih1, b_hh1,
                 W_ih2, b_ih2, b_hh2, b_out):
    """Host-side prep: shard over batch, gather embeddings, build transposed
    weight/bias layouts shared by all cores."""
    key = np.asarray(key, dtype=np.float32)
    value = np.asarray(value, dtype=np.float32)
    encoder_len = np.asarray(encoder_len)
    y = np.asarray(y)
    E = np.asarray(E, dtype=np.float32)

    # shifted inputs + embedding gather (host): (B, T) -> (B, T, EMBED)
    inputs = np.concatenate(
        [np.zeros((B, 1), dtype=y.dtype), y[:, :-1]], axis=1)
    embed = E[inputs]                                  # (B, T, EMBED)

    # LSTM weights, f-gate dropped (zero-state cell never uses it)
    def gate_sel(W, H):
        return np.concatenate([W[0:H], W[2 * H:3 * H], W[3 * H:4 * H]], axis=0)

    w1 = gate_sel(np.asarray(W_ih1, np.float32), H1)       # (1536, 256)
    w1T = np.ascontiguousarray(w1.T)                       # (256, 1536)
    bb1 = gate_sel((np.asarray(b_ih1, np.float32)
                    + np.asarray(b_hh1, np.float32))[:, None], H1)[:, 0]
    b1t = np.ascontiguousarray(bb1.reshape(12, 128).T)     # (128, 12)
    w2 = gate_sel(np.asarray(W_ih2, np.float32), KV)       # (384, 512)
    w2T = np.ascontiguousarray(w2.T)                       # (512, 384)
    bb2 = gate_sel((np.asarray(b_ih2, np.float32)
                    + np.asarray(b_hh2, np.float32))[:, None], KV)[:, 0]
    b2t = np.ascontiguousarray(bb2.reshape(3, 128).T)      # (128, 3)
    if LOGITS_DT == "f16":
        ET = np.ascontiguousarray(E.T).astype(np.float16)
    elif LOGITS_BF16:
        import ml_dtypes
        ET = np.ascontiguousarray(E.T).astype(ml_dtypes.bfloat16)
    else:
        ET = np.ascontiguousarray(E.T)                     # (256, 32000)
    boutt = np.ascontiguousarray(
        np.asarray(b_out, np.float32).reshape(NVT, 128).T)  # (128, 250)

    smask = (np.arange(S)[None, :] >= np.asarray(encoder_len)[:, None])
    maskb = np.where(smask, np.float32(-1e9), np.float32(0.0))  # (B, S)

    in_maps = []
    for c in range(N_CORES):
        bs = slice(c * BPC, (c + 1) * BPC)
        xT = np.ascontiguousarray(
            embed[bs].reshape(NTOK, EMBED).T)              # (256, 1024)
        keyT = np.ascontiguousarray(
            key[bs].transpose(0, 2, 1))                    # (4, 128, 512)
        in_maps.append({
            "ones1": np.ones((1, 128), np.float32),
            "ident": np.eye(128, dtype=np.float32),
            "xT": xT,
            "keyT": keyT,
            "val": np.ascontiguousarray(value[bs]),
            "maskb": np.ascontiguousarray(maskb[bs]),
            "w1T": w1T,
            "b1": b1t,
            "w2T": w2T,
            "b2": b2t,
            "ET": ET,
            "bout": boutt,
        })
    return in_maps


def _get_compiled():
    global _COMPILED
    if _COMPILED is None:
        _COMPILED = _build_module()
    return _COMPILED


def kernel(key, value, encoder_len, y, E, W_ih1, b_ih1, b_hh1,
           W_ih2, b_ih2, b_hh2, b_out):
    from concourse.bass_utils import run_bass_kernel_spmd

    nc = _get_compiled()
    in_maps = _prep_inputs(key, value, encoder_len, y, E, W_ih1, b_ih1, b_hh1,
                           W_ih2, b_ih2, b_hh2, b_out)
    res = run_bass_kernel_spmd(nc, in_maps, core_ids=list(range(N_CORES)))
    out = np.empty((B, VOCAB, T), np.float32)
    for c in range(N_CORES):
        # device layout (VOCAB, BPC*T) fp16 -> (BPC, VOCAB, T) fp32
        oc = np.asarray(res.results[c]["out"]).reshape(VOCAB, BPC, T)
        out[c * BPC:(c + 1) * BPC] = oc.transpose(1, 0, 2)
    return out



# revision 9
# speedup vs baseline: 430.0715x; 430.0715x over previous
"""Trainium2 Bass kernel for nn_Decoder (LSTM-cell decoder + dot-product
attention + tied-embedding projection), data-parallel over batch on 8 cores.

Contract: kernel(**inputs) takes the FULL unsharded numpy inputs (as produced
by the reference setup_inputs) and returns the FULL (B, VOCAB, T) fp32 output.

Strategy per core (4 batches each):
  - host: shift y, gather embeddings, transpose to (EMBED, ntok) layout
  - device: LSTM1/LSTM2 (zero-state cells; f-gate skipped), dot-product
    attention with length masking (mask added via K=1 ones-matmul into the
    PSUM accumulation group), softmax on free dim, PE-transpose of attn,
    context matmul, then the dominant (EMBED x VOCAB) tied projection with
    E^T streamed from HBM; all matmuls run as float32r (full-rate fp32).
"""
import sys, os

for _p in ("/opt/trn_rl_repo",):
    if _p not in sys.path and os.path.isdir(_p):
        sys.path.insert(0, _p)

import numpy as np

VOCAB = 32000
EMBED = 256
H1 = 512
KV = 128
B = 32
S = 512
T = 256
N_CORES = 8
BPC = B // N_CORES          # 4 batches per core
NTOK = BPC * T              # 1024 tokens per core
NVT = VOCAB // 128          # 250 vocab tiles
VCHUNK = 8                  # vocab tiles per E-stream chunk
NCHUNK = (NVT + VCHUNK - 1) // VCHUNK  # 32 (last chunk has 2 tiles)

# Logits-matmul dtype for the tied-projection phase. "f16" streams E^T as
# fp16 (h2/ctx cast to fp16): halves the dominant E-read DMA at full PE
# rate, and fp16's 11-bit mantissa keeps the end-to-end error ~3e-4
# (vs 2.2e-4 all-fp32r, 2.4e-3 bf16). "f32r" is the full-precision path.
LOGITS_DT = os.environ.get("KERNEL_LOGITS_DT", "f16")
LOGITS_BF16 = LOGITS_DT in ("bf16", "f16")
# Output store dtype. "f16" halves the dominant 131MB/core logits store
# (quantization adds ~1e-4 rel err, well under the 2e-2 gate); host widens
# back to fp32. Device layout is (VOCAB, BPC*T) so stores are contiguous;
# host transposes to (BPC, VOCAB, T).
OUT_DT = os.environ.get("KERNEL_OUT_DT", "f16")

_COMPILED = None


def _build_module(reps=0):
    """Build the kernel module. reps>0 wraps the whole body in a hardware
    For_i loop (used only for differential HW timing: one iteration == one
    full kernel execution including resident-input loads)."""
    import concourse.tile as tile
    import concourse.mybir as mybir
    from concourse import bacc, masks

    F32 = mybir.dt.float32
    F32R = mybir.dt.float32r
    BF16 = (mybir.dt.float16 if LOGITS_DT == "f16" else mybir.dt.bfloat16)
    EDT = BF16 if LOGITS_BF16 else F32R
    ODT = mybir.dt.float16 if OUT_DT == "f16" else F32
    AF = mybir.ActivationFunctionType
    AX = mybir.AxisListType
    ALU = mybir.AluOpType

    nc = bacc.Bacc("TRN2", target_bir_lowering=False, debug=False,
                   num_devices=N_CORES)

    def din(name, shape, dt=F32):
        return nc.dram_tensor(name, list(shape), dt, kind="ExternalInput").ap()

    xT_d = din("xT", (EMBED, NTOK), F32R)      # embedded tokens, transposed
    keyT_d = din("keyT", (BPC, KV, S), F32R)   # key[b].T
    val_d = din("val", (BPC, S, KV), F32R)     # value[b]
    mask_d = din("maskb", (BPC, S), F32R)      # 0 / -1e9 additive mask
    w1T_d = din("w1T", (EMBED, 3 * H1), F32R)  # W_ih1[i,g,o].T
    b1_d = din("b1", (128, 12))                # (b_ih1+b_hh1)[i,g,o] tiled
    w2T_d = din("w2T", (H1, 3 * KV), F32R)     # W_ih2[i,g,o].T
    b2_d = din("b2", (128, 3))
    ET_d = din("ET", (EMBED, VOCAB), EDT)      # E.T
    bout_d = din("bout", (128, NVT))           # b_out tiled
    ones_d = din("ones1", (1, 128), F32R)
    ident_d = din("ident", (128, 128), F32R)
    out_d = nc.dram_tensor("out", [VOCAB, NTOK], ODT,
                           kind="ExternalOutput").ap()

    with tile.TileContext(nc) as tc:
        import contextlib
        ctx = contextlib.ExitStack()
        with ctx:
            if reps:
                ctx.enter_context(tc.For_i(0, reps, 1, name="rep"))
            sb = ctx.enter_context(tc.tile_pool(name="sb", bufs=1))
            work = ctx.enter_context(tc.tile_pool(name="work", bufs=3))
            epool = ctx.enter_context(tc.tile_pool(name="epool", bufs=6))
            opool = ctx.enter_context(tc.tile_pool(name="opool", bufs=6))
            ps512 = ctx.enter_context(
                tc.tile_pool(name="ps512", bufs=4, space="PSUM"))
            pstr = ctx.enter_context(
                tc.tile_pool(name="pstr", bufs=2, space="PSUM"))
            psctx = ctx.enter_context(
                tc.tile_pool(name="psctx", bufs=2, space="PSUM"))

            # ---------------- resident inputs ----------------
            xT = [sb.tile([128, NTOK], F32R, name=f"xT{k}") for k in range(2)]
            for k in range(2):
                nc.sync.dma_start(xT[k][:], xT_d[k * 128:(k + 1) * 128, :])
            w1T = [sb.tile([128, 3 * H1], F32R, name=f"w1T{k}") for k in range(2)]
            for k in range(2):
                nc.sync.dma_start(w1T[k][:], w1T_d[k * 128:(k + 1) * 128, :])
            w2T = [sb.tile([128, 3 * KV], F32R, name=f"w2T{k}") for k in range(4)]
            for k in range(4):
                nc.sync.dma_start(w2T[k][:], w2T_d[k * 128:(k + 1) * 128, :])
            b1 = sb.tile([128, 12], F32)
            nc.sync.dma_start(b1[:], b1_d[:])
            b2 = sb.tile([128, 3], F32)
            nc.sync.dma_start(b2[:], b2_d[:])
            keyT = sb.tile([128, BPC * S], F32R)
            for b in range(BPC):
                nc.sync.dma_start(keyT[:, b * S:(b + 1) * S], keyT_d[b])
            valt = sb.tile([128, BPC * S], F32R)  # value k-tiles side by side
            for b in range(BPC):
                nc.sync.dma_start(
                    valt[:, b * S:(b + 1) * S].rearrange(
                        "p (st v) -> p st v", st=4),
                    val_d[b].rearrange("(st p) v -> p st v", p=128))
            maskb = sb.tile([1, BPC * S], F32R)
            nc.sync.dma_start(
                maskb[0:1, :].rearrange("p (b s) -> p b s", b=BPC),
                mask_d[None])
            bout = sb.tile([128, NVT], F32)
            nc.sync.dma_start(bout[:], bout_d[:])
            ones1 = sb.tile([1, 128], F32R)
            nc.sync.dma_start(ones1[:], ones_d[:])
            ident = sb.tile([128, 128], F32R)
            nc.sync.dma_start(ident[:], ident_d[:])

            h1T = [sb.tile([128, NTOK], F32R, name=f"h1T{k}") for k in range(4)]
            h2T = sb.tile([128, NTOK], F32R)
            ctxT = sb.tile([128, NTOK], EDT)
            h2L = (sb.tile([128, NTOK], BF16, name="h2L")
                   if LOGITS_BF16 else h2T)

            def r(ap):
                return ap if ap.dtype == F32R else ap.bitcast(F32R)

            # ---------------- LSTM 1 ----------------
            # gates^T tiles: m = 0..3 -> i, 4..7 -> g, 8..11 -> o
            for n in range(2):
                tok = slice(n * 512, (n + 1) * 512)
                for msub in range(4):
                    pg = {}
                    for gi, gname in enumerate(("i", "g", "o")):
                        m = gi * 4 + msub
                        ps = ps512.tile([128, 512], F32, name="ps_mm",
                                        tag="ps_mm")
                        for k in range(2):
                            nc.tensor.matmul(
                                ps[:],
                                r(w1T[k][:, m * 128:(m + 1) * 128]),
                                r(xT[k][:, tok]),
                                start=(k == 0), stop=(k == 1))
                        pg[gname] = (ps, m)
                    sig_i = work.tile([128, 512], F32, tag="lstm_act")
                    nc.scalar.activation(sig_i[:], pg["i"][0][:], AF.Sigmoid,
                                         bias=b1[:, pg["i"][1]:pg["i"][1] + 1])
                    tanh_g = work.tile([128, 512], F32, tag="lstm_act")
                    nc.scalar.activation(tanh_g[:], pg["g"][0][:], AF.Tanh,
                                         bias=b1[:, pg["g"][1]:pg["g"][1] + 1])
                    cst = work.tile([128, 512], F32, tag="lstm_act")
                    nc.vector.tensor_mul(cst[:], sig_i[:], tanh_g[:])
                    tanh_c = work.tile([128, 512], F32, tag="lstm_act")
                    nc.scalar.activation(tanh_c[:], cst[:], AF.Tanh)
                    sig_o = work.tile([128, 512], F32, tag="lstm_act")
                    nc.scalar.activation(sig_o[:], pg["o"][0][:], AF.Sigmoid,
                                         bias=b1[:, pg["o"][1]:pg["o"][1] + 1])
                    nc.vector.tensor_mul(h1T[msub][:, tok], sig_o[:], tanh_c[:])

            # ---------------- LSTM 2 ----------------
            for n in range(2):
                tok = slice(n * 512, (n + 1) * 512)
                pg = {}
                for gi, gname in enumerate(("i", "g", "o")):
                    ps = ps512.tile([128, 512], F32, name="ps_mm", tag="ps_mm")
                    for k in range(4):
                        nc.tensor.matmul(
                            ps[:],
                            r(w2T[k][:, gi * 128:(gi + 1) * 128]),
                            r(h1T[k][:, tok]),
                            start=(k == 0), stop=(k == 3))
                    pg[gname] = ps
                sig_i = work.tile([128, 512], F32, tag="lstm_act")
                nc.scalar.activation(sig_i[:], pg["i"][:], AF.Sigmoid,
                                     bias=b2[:, 0:1])
                tanh_g = work.tile([128, 512], F32, tag="lstm_act")
                nc.scalar.activation(tanh_g[:], pg["g"][:], AF.Tanh,
                                     bias=b2[:, 1:2])
                cst = work.tile([128, 512], F32, tag="lstm_act")
                nc.vector.tensor_mul(cst[:], sig_i[:], tanh_g[:])
                tanh_c = work.tile([128, 512], F32, tag="lstm_act")
                nc.scalar.activation(tanh_c[:], cst[:], AF.Tanh)
                sig_o = work.tile([128, 512], F32, tag="lstm_act")
                nc.scalar.activation(sig_o[:], pg["o"][:], AF.Sigmoid,
                                     bias=b2[:, 2:3])
                nc.vector.tensor_mul(h2T[:, tok], sig_o[:], tanh_c[:])
                if LOGITS_BF16:
                    nc.vector.tensor_copy(h2L[:, tok], h2T[:, tok])

            # ---------------- attention ----------------
            for b in range(BPC):
                attnT = [work.tile([128, T], F32R, tag=f"attnT{st}",
                                   name=f"attnT{st}")
                         for st in range(4)]
                for tt in range(2):
                    tcol = b * T + tt * 128
                    ps_e = ps512.tile([128, 512], F32, name="ps_mm",
                                      tag="ps_mm")
                    nc.tensor.matmul(ps_e[:],
                                     r(h2T[:, tcol:tcol + 128]),
                                     r(keyT[:, b * S:(b + 1) * S]),
                                     start=True, stop=False)
                    nc.tensor.matmul(ps_e[:], r(ones1[:]),
                                     r(maskb[0:1, b * S:(b + 1) * S]),
                                     start=False, stop=True)
                    negmax = work.tile([128, 1], F32, tag="stat")
                    nc.vector.tensor_reduce(negmax[:], ps_e[:], axis=AX.X,
                                            op=ALU.max, negate=True)
                    attn = work.tile([128, 512], F32, tag="attn")
                    rowsum = work.tile([128, 1], F32, tag="stat")
                    nc.scalar.activation(attn[:], ps_e[:], AF.Exp,
                                         bias=negmax[:, 0:1],
                                         accum_out=rowsum[:, 0:1])
                    recip = work.tile([128, 1], F32, tag="stat")
                    nc.vector.reciprocal(recip[:], rowsum[:])
                    attn_n = work.tile([128, 512], F32R, tag="attn_n")
                    nc.vector.tensor_scalar_mul(attn_n[:], attn[:],
                                                recip[:, 0:1])
                    for st in range(4):
                        ps_t = pstr.tile([128, 128], F32R, name="ps_tr",
                                         tag="ps_tr")
                        nc.tensor.transpose(ps_t[:],
                                            r(attn_n[:, st * 128:(st + 1) * 128]),
                                            r(ident[:]))
                        dst = attnT[st][:, tt * 128:(tt + 1) * 128]
                        if st % 2 == 0:
                            nc.scalar.copy(dst, ps_t[:])
                        else:
                            nc.vector.tensor_copy(dst, ps_t[:])
                ps_c = psctx.tile([128, T], F32, name="ps_ctx", tag="ps_ctx")
                for st in range(4):
                    nc.tensor.matmul(
                        ps_c[:],
                        r(valt[:, (b * 4 + st) * 128:(b * 4 + st + 1) * 128]),
                        r(attnT[st][:]),
                        start=(st == 0), stop=(st == 3))
                if b % 2 == 0:
                    nc.scalar.copy(ctxT[:, b * T:(b + 1) * T], ps_c[:])
                else:
                    nc.vector.tensor_copy(ctxT[:, b * T:(b + 1) * T], ps_c[:])

            # ---------------- logits: out = [h2; ctx]^T . [E_lo; E_hi] ----
            for ci in range(NCHUNK):
                nv = min(VCHUNK, NVT - ci * VCHUNK)
                cols = nv * 128
                base = ci * VCHUNK * 128
                ea = epool.tile([128, VCHUNK * 128], EDT, tag="ea")
                eb = epool.tile([128, VCHUNK * 128], EDT, tag="eb")
                nc.scalar.dma_start(ea[:, :cols],
                                    ET_d[0:128, base:base + cols])
                nc.scalar.dma_start(eb[:, :cols],
                                    ET_d[128:256, base:base + cols])
                for j in range(nv):
                    v = ci * VCHUNK + j
                    osb = opool.tile([128, NTOK], ODT, tag="osb")
                    for half in range(2):
                        tok = slice(half * 512, (half + 1) * 512)
                        ps_l = ps512.tile([128, 512], F32, name="ps_mm",
                                          tag="ps_mm")
                        nc.tensor.matmul(ps_l[:],
                                         ea[:, j * 128:(j + 1) * 128],
                                         h2L[:, tok],
                                         start=True, stop=False)
                        nc.tensor.matmul(ps_l[:],
                                         eb[:, j * 128:(j + 1) * 128],
                                         ctxT[:, tok],
                                         start=False, stop=True)
                        if half == 0:
                            nc.scalar.activation(osb[:, tok], ps_l[:],
                                                 AF.Identity,
                                                 bias=bout[:, v:v + 1])
                        else:
                            nc.vector.tensor_scalar_add(osb[:, tok], ps_l[:],
                                                        bout[:, v:v + 1])
                    nc.sync.dma_start(out_d[v * 128:(v + 1) * 128, :],
                                      osb[:])

    nc.compile()
    return nc


def _prep_inputs(key, value, encoder_len, y, E, W_ih1, b_ih1, b_hh1,
                 W_ih2, b_ih2, b_hh2, b_out):
    """Host-side prep: shard over batch, gather embeddings, build transposed
    weight/bias layouts shared by all cores."""
    key = np.asarray(key, dtype=np.float32)
    value = np.asarray(value, dtype=np.float32)
    encoder_len = np.asarray(encoder_len)
    y = np.asarray(y)
    E = np.asarray(E, dtype=np.float32)

    # shifted inputs + embedding gather (host): (B, T) -> (B, T, EMBED)
    inputs = np.concatenate(
        [np.zeros((B, 1), dtype=y.dtype), y[:, :-1]], axis=1)
    embed = E[inputs]                                  # (B, T, EMBED)

    # LSTM weights, f-gate dropped (zero-state cell never uses it)
    def gate_sel(W, H):
        return np.concatenate([W[0:H], W[2 * H:3 * H], W[3 * H:4 * H]], axis=0)

    w1 = gate_sel(np.asarray(W_ih1, np.float32), H1)       # (1536, 256)
    w1T = np.ascontiguousarray(w1.T)                       # (256, 1536)
    bb1 = gate_sel((np.asarray(b_ih1, np.float32)
                    + np.asarray(b_hh1, np.float32))[:, None], H1)[:, 0]
    b1t = np.ascontiguousarray(bb1.reshape(12, 128).T)     # (128, 12)
    w2 = gate_sel(np.asarray(W_ih2, np.float32), KV)       # (384, 512)
    w2T = np.ascontiguousarray(w2.T)                       # (512, 384)
    bb2 = gate_sel((np.asarray(b_ih2, np.float32)
                    + np.asarray(b_hh2, np.float32))[:, None], KV)[:, 0]
    b2t = np.ascontiguousarray(bb2.reshape(3, 128).T)      # (128, 3)
    if LOGITS_DT == "f16":
        ET = np.ascontiguousarray(E.T).astype(np.float16)
    elif LOGITS_BF16:
        import ml_dtypes
        ET = np.ascontiguousarray(E.T).astype(ml_dtypes.bfloat16)
    else:
        ET = np.ascontiguousarray(E.T)                     # (256, 32000)
    boutt = np.ascontiguousarray(
        np.asarray(b_out, np.float32).reshape(NVT, 128).T)  # (128, 250)

    smask = (np.arange(S)[None, :] >= np.asarray(encoder_len)[:, None])
    maskb = np.where(smask, np.float32(-1e9), np.float32(0.0))  # (B, S)

    in_maps = []
    for c in range(N_CORES):
        bs = slice(c * BPC, (c + 1) * BPC)
        xT = np.ascontiguousarray(
            embed[bs].reshape(NTOK, EMBED).T)              # (256, 1024)
        keyT = np.ascontiguousarray(
            key[bs].transpose(0, 2, 1))                    # (4, 128, 512)
        in_maps.append({
            "ones1": np.ones((1, 128), np.float32),
            "ident": np.eye(128, dtype=np.float32),
            "xT": xT,
            "keyT": keyT,
            "val": np.ascontiguousarray(value[bs]),
            "maskb": np.ascontiguousarray(maskb[bs]),
            "w1T": w1T,
            "b1": b1t,
            "w2T": w2T,
            "b2": b2t,
            "ET": ET,
            "bout": boutt,
        })
    return in_maps


def _get_compiled(reps=0):
    global _COMPILED
    if reps:
        return _build_module(reps)
    if _COMPILED is None:
        _COMPILED = _build_module()
    return _COMPILED


def kernel(key, value, encoder_len, y, E, W_ih1, b_ih1, b_hh1,
           W_ih2, b_ih2, b_hh2, b_out):
    from concourse.bass_utils import run_bass_kernel_spmd

    nc = _get_compiled()
    in_maps = _prep_inputs(key, value, encoder_len, y, E, W_ih1, b_ih1, b_hh1,
                           W_ih2, b_ih2, b_hh2, b_out)
    res = run_bass_kernel_spmd(nc, in_maps, core_ids=list(range(N_CORES)))
    out = np.empty((B, VOCAB, T), np.float32)
    for c in range(N_CORES):
        # device layout (VOCAB, BPC*T) fp16 -> (BPC, VOCAB, T) fp32
        oc = np.asarray(res.results[c]["out"]).reshape(VOCAB, BPC, T)
        out[c * BPC:(c + 1) * BPC] = oc.transpose(1, 0, 2)
    return out

